# revision 11
# baseline (speedup 1.0000x reference)
"""Trainium2 Bass kernel for nn_CrossAttentionFusion (cross-attention + BitLinear FFN).

Sharding: 8 cores = 4 batches x 2 sequence-halves. Each core:
  - owns 1024 query tokens (sem shard, feature-major),
  - computes K/V for its batch's full 2048 tokens from pro (feature-major),
  - runs full attention for its queries + BitLinear FFN, writes its out^T shard.
No collectives needed; host does all layout transposes and the final gather.
"""
import math
import numpy as np
from contextlib import ExitStack

import concourse.bass as bass
import concourse.bass_isa as bass_isa
import concourse.tile as tile
from concourse import bacc, mybir
from concourse.bass_utils import run_bass_kernel_spmd

F32 = mybir.dt.float32
BF16 = mybir.dt.bfloat16
AF = mybir.ActivationFunctionType
ALU = mybir.AluOpType

B, S, DS, DP, H = 4, 2048, 1024, 512, 8
DF = 4 * DS
HD = DS // H          # 128
TOK = 1024            # query tokens per core
N_CORES = 8
EPS = 1e-6
C_RND = 12582912.0    # 1.5 * 2**23 : +C-C = round-to-nearest-even
QK_SCALE = 1.0 / math.sqrt(HD)

P = 128
M_SEM = DS // P       # 8
M_PRO = DP // P       # 4
M_FF = DF // P        # 32
NT_Q = TOK // 512     # 2
NT_K = S // P         # 16
MT_V = S // P         # 16


def build_nc(debug_outs=False):
    nc = bacc.Bacc("TRN2", target_bir_lowering=False, debug=False, num_devices=N_CORES)

    semT = nc.dram_tensor("semT", [DS, TOK], F32, kind="ExternalInput").ap()
    proT = nc.dram_tensor("proT", [DP, S], F32, kind="ExternalInput").ap()
    wqT = nc.dram_tensor("wqT", [DS, DS], BF16, kind="ExternalInput").ap()
    wkT = nc.dram_tensor("wkT", [DP, DS], BF16, kind="ExternalInput").ap()
    wvT = nc.dram_tensor("wvT", [DP, DS], BF16, kind="ExternalInput").ap()
    woT = nc.dram_tensor("woT", [DS, DS], BF16, kind="ExternalInput").ap()
    w1T = nc.dram_tensor("w1T", [DS, DF], F32, kind="ExternalInput").ap()
    w2T = nc.dram_tensor("w2T", [DF, DS], F32, kind="ExternalInput").ap()
    w1s = nc.dram_tensor("w1s", [P, DF], F32, kind="ExternalInput").ap()
    w2s = nc.dram_tensor("w2s", [DP, DS], F32, kind="ExternalInput").ap()
    gsem = nc.dram_tensor("gsem", [P, M_SEM], F32, kind="ExternalInput").ap()
    gpro = nc.dram_tensor("gpro", [P, M_PRO], F32, kind="ExternalInput").ap()
    gff = nc.dram_tensor("gff", [P, M_SEM], F32, kind="ExternalInput").ap()
    bq = nc.dram_tensor("bq", [P, M_SEM], F32, kind="ExternalInput").ap()
    bk = nc.dram_tensor("bk", [P, M_SEM], F32, kind="ExternalInput").ap()
    bv = nc.dram_tensor("bv", [P, M_SEM], F32, kind="ExternalInput").ap()
    bo = nc.dram_tensor("bo", [P, M_SEM], F32, kind="ExternalInput").ap()
    alpha = nc.dram_tensor("alpha", [P, M_FF], F32, kind="ExternalInput").ap()
    beta = nc.dram_tensor("beta", [P, M_FF], F32, kind="ExternalInput").ap()
    outT = nc.dram_tensor("outT", [DS, TOK], F32, kind="ExternalOutput").ap()

    dbg = {}
    if debug_outs:
        for name, shape, dt in [
            ("dbg_semn", [DS, TOK], BF16), ("dbg_q", [DS, TOK], BF16),
            ("dbg_k", [DS, S], BF16), ("dbg_v", [S, DS], BF16),
            ("dbg_ctx", [DS, TOK], BF16), ("dbg_semout", [DS, TOK], F32),
            ("dbg_xq", [DS, TOK], BF16), ("dbg_h", [DF, TOK], BF16),
            ("dbg_hq", [DF, TOK], BF16), ("dbg_mw", [1, 2], F32),
        ]:
            dbg[name] = nc.dram_tensor(name, shape, dt, kind="ExternalOutput").ap()

    with tile.TileContext(nc) as tc, ExitStack() as top:
        persist = top.enter_context(tc.tile_pool(name="persist", bufs=1))
        rows = top.enter_context(tc.tile_pool(name="rows", bufs=1))
        ps_mm = top.enter_context(tc.tile_pool(name="ps_mm", bufs=2, space="PSUM"))
        ps_s = top.enter_context(tc.tile_pool(name="ps_s", bufs=2, space="PSUM"))
        ps_ctx = top.enter_context(tc.tile_pool(name="ps_ctx", bufs=2, space="PSUM"))
        ps_row = top.enter_context(tc.tile_pool(name="ps_row", bufs=1, space="PSUM"))

        ones = persist.tile([P, 1], BF16)
        nc.vector.memset(ones[:], 1.0)
        ones_row = persist.tile([1, P], BF16)
        nc.vector.memset(ones_row[:], 1.0)
        eps_t = persist.tile([1, 1], F32)
        nc.vector.memset(eps_t[:], EPS)

        gsem_sb = persist.tile([P, M_SEM], F32)
        gpro_sb = persist.tile([P, M_PRO], F32)
        gff_sb = persist.tile([P, M_SEM], F32)
        bq_sb = persist.tile([P, M_SEM], F32)
        bk_sb = persist.tile([P, M_SEM], F32)
        bv_sb = persist.tile([P, M_SEM], F32)
        bo_sb = persist.tile([P, M_SEM], F32)
        alpha_sb = persist.tile([P, M_FF], F32)
        rbeta_sb = persist.tile([P, M_FF], F32)
        for ap_d, t in [(gsem, gsem_sb), (gpro, gpro_sb), (gff, gff_sb),
                        (bq, bq_sb), (bk, bk_sb), (bv, bv_sb), (bo, bo_sb),
                        (alpha, alpha_sb)]:
            nc.sync.dma_start(t[:], ap_d[:])
        beta_t = persist.tile([P, M_FF], F32)
        nc.sync.dma_start(beta_t[:], beta[:])
        nc.vector.tensor_scalar(rbeta_sb[:], beta_t[:], 1e-9, None, ALU.add)
        nc.vector.reciprocal(rbeta_sb[:], rbeta_sb[:])

        semT_r = semT.rearrange("(m p) t -> p m t", p=P)

        def rmsnorm_fm(pool, fetch, nm, T, g_sb, out_bf):
            """feature-major rmsnorm: out_bf[:, m, :] = x_m * g_m * rsqrt(ms+eps)"""
            D = nm * P
            rs_row = rows.tile([1, T], F32, tag="rs_row")
            xs = [fetch(m) for m in range(nm)]
            for ch in range(T // 512):
                ps = ps_row.tile([1, 512], F32, tag="psrow")
                for m in range(nm):
                    sq = pool.tile([P, 512], BF16, tag="sq", bufs=3)
                    nc.scalar.activation(sq[:], xs[m][:, ch * 512:(ch + 1) * 512],
                                         AF.Square)
                    nc.tensor.matmul(ps[:], ones[:], sq[:],
                                     start=(m == 0), stop=(m == nm - 1))
                nc.scalar.activation(rs_row[:, ch * 512:(ch + 1) * 512], ps[:],
                                     AF.Ln, bias=eps_t[:], scale=1.0 / D)
            nc.scalar.activation(rs_row[:], rs_row[:], AF.Exp, scale=-0.5)
            rs_bc = pool.tile([P, T], F32, tag="rs_bc", bufs=1)
            nc.gpsimd.partition_broadcast(rs_bc[:], rs_row[:])
            for m in range(nm):
                nc.vector.scalar_tensor_tensor(
                    out=out_bf[:, m, :], in0=xs[m][:],
                    scalar=g_sb[:, m:m + 1], in1=rs_bc[:],
                    op0=ALU.mult, op1=ALU.mult)

        # ================= phase 1: input norms =================
        es_norm = ExitStack()
        pnorm = es_norm.enter_context(tc.tile_pool(name="pnorm", bufs=1))
        semn_sb = pnorm.tile([P, M_SEM, TOK], BF16)
        pron_sb = pnorm.tile([P, M_PRO, S], BF16)

        with tc.tile_pool(name="pin1", bufs=1) as pin1:
            semT_sb = pin1.tile([P, M_SEM, TOK], F32)
            nc.sync.dma_start(semT_sb[:], semT_r)
            rmsnorm_fm(pin1, lambda m: semT_sb[:, m, :], M_SEM, TOK, gsem_sb, semn_sb)

        with tc.tile_pool(name="pin2", bufs=1) as pin2:
            proT_sb = pin2.tile([P, M_PRO, S], F32)
            nc.sync.dma_start(proT_sb[:], proT.rearrange("(m p) t -> p m t", p=P))
            rmsnorm_fm(pin2, lambda m: proT_sb[:, m, :], M_PRO, S, gpro_sb, pron_sb)

        if debug_outs:
            nc.sync.dma_start(dbg["dbg_semn"].rearrange("(m p) t -> p m t", p=P),
                              semn_sb[:])

        # ===== phase 2: mean(|w|) via per-core strips + AllReduce =====
        with tc.tile_pool(name="pwmean", bufs=2) as pwm, \
             tc.tile_pool(name="pdram", bufs=1, space="DRAM") as pdram:
            def strip_sum(ws_ap, nrows, cols, name):
                ntile = nrows // P
                mcols = rows.tile([P, ntile], F32, tag=f"mcols_{name}")
                for j in range(ntile):
                    wt = pwm.tile([P, DF], F32, tag="wmean")
                    nc.sync.dma_start(wt[:, :cols], ws_ap[j * P:(j + 1) * P, :])
                    nc.scalar.activation(wt[:, :cols], wt[:, :cols], AF.Abs,
                                         accum_out=mcols[:, j:j + 1])
                msum = rows.tile([P, 1], F32, tag=f"msum_{name}")
                nc.vector.tensor_reduce(msum[:], mcols[:], axis=mybir.AxisListType.X,
                                        op=ALU.add)
                msum_all = rows.tile([P, 1], F32, tag=f"msuma_{name}")
                nc.gpsimd.partition_all_reduce(msum_all[:], msum[:], P,
                                               bass_isa.ReduceOp.add)
                return msum_all

            s1 = strip_sum(w1s, P, DF, "w1")
            s2 = strip_sum(w2s, DP, DS, "w2")
            loc = rows.tile([1, 2], F32, tag="ccloc")
            nc.vector.tensor_copy(loc[:, 0:1], s1[0:1, :])
            nc.vector.tensor_copy(loc[:, 1:2], s2[0:1, :])
            cin = pdram.tile([1, 2], F32)
            cout = pdram.tile([1, 2], F32)
            nc.sync.dma_start(cin[:], loc[:])
            nc.gpsimd.collective_compute(
                "AllReduce", ALU.add,
                replica_groups=[list(range(N_CORES))],
                ins=[cin.opt()], outs=[cout.opt()])
            tot = rows.tile([1, 2], F32, tag="cctot")
            nc.sync.dma_start(tot[:], cout[:])
            mwrow = rows.tile([1, 2], F32, tag="mwrow")
            nc.vector.tensor_scalar(mwrow[:, 0:1], tot[:, 0:1], 1.0 / (DS * DF),
                                    None, ALU.mult)
            nc.vector.tensor_scalar(mwrow[:, 1:2], tot[:, 1:2], 1.0 / (DF * DS),
                                    None, ALU.mult)
            mw_all = rows.tile([P, 2], F32, tag="mwall")
            nc.gpsimd.partition_broadcast(mw_all[:], mwrow[:])
            mw1, mw2 = mw_all[:, 0:1], mw_all[:, 1:2]
            sw_all = rows.tile([P, 2], F32, tag="swall")
            nc.vector.reciprocal(sw_all[:], mw_all[:])
            sw1_bc, sw2_bc = sw_all[:, 0:1], sw_all[:, 1:2]
        if debug_outs:
            nc.sync.dma_start(dbg["dbg_mw"][:], mwrow[:])

        # ================= phase 3: Q/K/V =================
        es_qkv = ExitStack()
        pqkv = es_qkv.enter_context(tc.tile_pool(name="pqkv", bufs=1, side="right"))
        q_sb = pqkv.tile([P, M_SEM, TOK], BF16)
        k_sb = pqkv.tile([P, M_SEM, S], BF16)
        v_sb = pqkv.tile([P, MT_V, DS], BF16)

        with tc.tile_pool(name="pw3", bufs=1) as pw3:
            wq_sb = pw3.tile([P, M_SEM, DS], BF16)
            nc.sync.dma_start(wq_sb[:], wqT.rearrange("(m p) o -> p m o", p=P))
            for m in range(M_SEM):
                for n in range(NT_Q):
                    ps = ps_mm.tile([P, 512], F32, tag="mm")
                    for kk in range(M_SEM):
                        nc.tensor.matmul(ps[:], wq_sb[:, kk, m * P:(m + 1) * P],
                                         semn_sb[:, kk, n * 512:(n + 1) * 512],
                                         start=(kk == 0), stop=(kk == M_SEM - 1))
                    nc.scalar.activation(q_sb[:, m, n * 512:(n + 1) * 512], ps[:],
                                         AF.Identity, bias=bq_sb[:, m:m + 1])

            wk_sb = pw3.tile([P, M_PRO, DS], BF16)
            nc.sync.dma_start(wk_sb[:], wkT.rearrange("(m p) o -> p m o", p=P))
            for m in range(M_SEM):
                for n in range(S // 512):
                    ps = ps_mm.tile([P, 512], F32, tag="mm")
                    for kk in range(M_PRO):
                        nc.tensor.matmul(ps[:], wk_sb[:, kk, m * P:(m + 1) * P],
                                         pron_sb[:, kk, n * 512:(n + 1) * 512],
                                         start=(kk == 0), stop=(kk == M_PRO - 1))
                    nc.scalar.activation(k_sb[:, m, n * 512:(n + 1) * 512], ps[:],
                                         AF.Identity, bias=bk_sb[:, m:m + 1])

            wv_sb = pw3.tile([P, M_PRO, DS], BF16)
            nc.sync.dma_start(wv_sb[:], wvT.rearrange("(m p) o -> p m o", p=P))
            for mt in range(MT_V):
                for n in range(DS // 512):
                    ps = ps_mm.tile([P, 512], F32, tag="mm")
                    for kk in range(M_PRO):
                        nc.tensor.matmul(ps[:], pron_sb[:, kk, mt * P:(mt + 1) * P],
                                         wv_sb[:, kk, n * 512:(n + 1) * 512],
                                         start=(kk == 0), stop=(kk == M_PRO - 1))
                    # bias bv folded in at ctx evac
                    nc.scalar.activation(v_sb[:, mt, n * 512:(n + 1) * 512], ps[:],
                                         AF.Copy)
        es_norm.close()   # semn/pron freed

        if debug_outs:
            nc.sync.dma_start(dbg["dbg_q"].rearrange("(m p) t -> p m t", p=P), q_sb[:])
            nc.sync.dma_start(dbg["dbg_k"].rearrange("(m p) t -> p m t", p=P), k_sb[:])
            nc.sync.dma_start(dbg["dbg_v"].rearrange("(m p) t -> p m t", p=P), v_sb[:])

        # ================= phase 4: attention =================
        es_ctx = ExitStack()
        pctx = es_ctx.enter_context(tc.tile_pool(name="pctx", bufs=1))
        ctx_sb = pctx.tile([P, M_SEM, TOK], BF16)

        with tc.tile_pool(name="pattn", bufs=1) as pattn:
            for h in range(H):
                for n in range(NT_Q):
                    pt = pattn.tile([P, NT_K, 512], BF16, tag="ptile", bufs=2)
                    den_ps = ps_row.tile([1, 512], F32, tag="psrow")
                    for mt in range(NT_K):
                        ps = ps_s.tile([P, 512], F32, tag="sps")
                        nc.tensor.matmul(ps[:], k_sb[:, h, mt * P:(mt + 1) * P],
                                         q_sb[:, h, n * 512:(n + 1) * 512],
                                         start=True, stop=True)
                        nc.scalar.activation(pt[:, mt, :], ps[:], AF.Exp,
                                             scale=QK_SCALE)
                        nc.tensor.matmul(den_ps[:], ones[:], pt[:, mt, :],
                                         start=(mt == 0), stop=(mt == NT_K - 1))
                    rden_row = rows.tile([1, 512], F32, tag="rdenrow")
                    nc.vector.reciprocal_approx_fast(rden_row[:], den_ps[:])
                    rden16 = rows.tile([1, 512], BF16, tag="rden16")
                    nc.vector.tensor_copy(rden16[:], rden_row[:])
                    bc_ps = ps_s.tile([P, 512], F32, tag="bcast", bufs=1)
                    nc.tensor.matmul(bc_ps[:], ones_row[:], rden16[:],
                                     start=True, stop=True)
                    rden_bc = pattn.tile([P, 512], F32, tag="rdenbc", bufs=2)
                    nc.scalar.activation(rden_bc[:], bc_ps[:], AF.Copy)
                    cps = ps_ctx.tile([P, 512], F32, tag="ctxps")
                    for mt in range(NT_K):
                        nc.tensor.matmul(cps[:], v_sb[:, mt, h * P:(h + 1) * P],
                                         pt[:, mt, :],
                                         start=(mt == 0), stop=(mt == NT_K - 1))
                    tnorm = pattn.tile([P, 512], F32, tag="ctxnorm", bufs=2)
                    nc.vector.tensor_tensor(tnorm[:], cps[:], rden_bc[:], op=ALU.mult)
                    nc.vector.tensor_scalar(ctx_sb[:, h, n * 512:(n + 1) * 512],
                                            tnorm[:], bv_sb[:, h:h + 1], None,
                                            ALU.add)
        es_qkv.close()   # q/k/v freed

        if debug_outs:
            nc.sync.dma_start(dbg["dbg_ctx"].rearrange("(m p) t -> p m t", p=P),
                              ctx_sb[:])

        # ================= phase 5: out-proj + residual =================
        es_so = ExitStack()
        psem = es_so.enter_context(tc.tile_pool(name="psem", bufs=1, side="right"))
        semout_sb = psem.tile([P, M_SEM, TOK], F32)

        with tc.tile_pool(name="pw5", bufs=1) as pw5:
            wo_sb = pw5.tile([P, M_SEM, DS], BF16)
            nc.sync.dma_start(wo_sb[:], woT.rearrange("(m p) o -> p m o", p=P))
            for m in range(M_SEM):
                semres = pw5.tile([P, TOK], F32, tag="semres", bufs=2)
                nc.sync.dma_start(semres[:], semT_r[:, m, :])
                for n in range(NT_Q):
                    ps = ps_mm.tile([P, 512], F32, tag="mm")
                    for kk in range(M_SEM):
                        nc.tensor.matmul(ps[:], wo_sb[:, kk, m * P:(m + 1) * P],
                                         ctx_sb[:, kk, n * 512:(n + 1) * 512],
                                         start=(kk == 0), stop=(kk == M_SEM - 1))
                    t = pw5.tile([P, 512], F32, tag="oproj", bufs=3)
                    nc.scalar.activation(t[:], ps[:], AF.Identity,
                                         bias=bo_sb[:, m:m + 1])
                    nc.vector.tensor_tensor(
                        semout_sb[:, m, n * 512:(n + 1) * 512], t[:],
                        semres[:, n * 512:(n + 1) * 512], op=ALU.add)
        es_ctx.close()   # ctx freed

        if debug_outs:
            nc.sync.dma_start(dbg["dbg_semout"].rearrange("(m p) t -> p m t", p=P),
                              semout_sb[:])

        # ================= phase 6: ff-norm + act_quant(x) =================
        es_xq = ExitStack()
        pxq = es_xq.enter_context(tc.tile_pool(name="pxq", bufs=1))
        xq_sb = pxq.tile([P, M_SEM, TOK], BF16)
        sx_bc = pxq.tile([P, TOK], F32)
        deq1_bc = pxq.tile([P, TOK], F32)

        with tc.tile_pool(name="pff6", bufs=1) as pff6:
            xn_sb = pff6.tile([P, M_SEM, TOK], BF16)
            rmsnorm_fm(pff6, lambda m: semout_sb[:, m, :], M_SEM, TOK, gff_sb, xn_sb)

            mx = pff6.tile([P, TOK], BF16, tag="mx")
            mn = pff6.tile([P, TOK], BF16, tag="mn")
            nc.vector.tensor_tensor(mx[:], xn_sb[:, 0, :], xn_sb[:, 1, :], op=ALU.max)
            nc.vector.tensor_tensor(mn[:], xn_sb[:, 0, :], xn_sb[:, 1, :], op=ALU.min)
            for m in range(2, M_SEM):
                nc.vector.tensor_tensor(mx[:], mx[:], xn_sb[:, m, :], op=ALU.max)
                nc.vector.tensor_tensor(mn[:], mn[:], xn_sb[:, m, :], op=ALU.min)
            am = pff6.tile([P, TOK], BF16, tag="amax")
            nc.vector.scalar_tensor_tensor(out=am[:], in0=mn[:], scalar=-1.0,
                                           in1=mx[:], op0=ALU.mult, op1=ALU.max)
            amc = pff6.tile([P, TOK], F32, tag="amc")
            nc.gpsimd.partition_all_reduce(amc[:], am[:], P, bass_isa.ReduceOp.absmax)
            nc.vector.tensor_scalar(amc[:], amc[:], 1e-5, None, ALU.max)
            nc.vector.reciprocal_approx_fast(sx_bc[:], amc[:])
            nc.vector.tensor_scalar(sx_bc[:], sx_bc[:], 127.0, None, ALU.mult)
            nc.vector.tensor_scalar(deq1_bc[:], amc[:], mw1, 1.0 / 127.0,
                                    ALU.mult, ALU.mult)

            for m in range(M_SEM):
                t = pff6.tile([P, TOK], F32, tag="xqt", bufs=2)
                nc.vector.tensor_tensor(t[:], xn_sb[:, m, :], sx_bc[:], op=ALU.mult)
                nc.vector.tensor_scalar(xq_sb[:, m, :], t[:], C_RND, C_RND, ALU.add,
                                        ALU.subtract)
        if debug_outs:
            nc.sync.dma_start(dbg["dbg_xq"].rearrange("(m p) t -> p m t", p=P),
                              xq_sb[:])

        # ================= phase 7: FFN1 + SnakeBeta =================
        es_h = ExitStack()
        ph = es_h.enter_context(tc.tile_pool(name="ph", bufs=1, side="right"))
        h_sb = ph.tile([P, M_FF, TOK], BF16)    # 64 KB/p

        with tc.tile_pool(name="pff7", bufs=1) as pff7:
            w1r = w1T.rearrange("(kt p) o -> p kt o", p=P)
            for m in range(M_FF):
                wc = pff7.tile([P, M_SEM, P], F32, tag="w1c", bufs=2)
                nc.sync.dma_start(wc[:], w1r[:, :, m * P:(m + 1) * P])
                w1q = pff7.tile([P, M_SEM, P], BF16, tag="w1q", bufs=2)
                tw = pff7.tile([P, M_SEM * P], F32, tag="terntmp", bufs=2)
                wcf = wc[:].rearrange("p a b -> p (a b)")
                nc.vector.tensor_scalar(tw[:], wcf, sw1_bc, 1.49, ALU.mult,
                                        ALU.min)
                nc.vector.tensor_scalar(tw[:], tw[:], -1.49, C_RND, ALU.max, ALU.add)
                nc.vector.tensor_scalar(w1q[:].rearrange("p a b -> p (a b)"), tw[:],
                                        C_RND, None, ALU.subtract)
                for n in range(NT_Q):
                    ps = ps_mm.tile([P, 512], F32, tag="mm")
                    for kk in range(M_SEM):
                        nc.tensor.matmul(ps[:], w1q[:, kk, :],
                                         xq_sb[:, kk, n * 512:(n + 1) * 512],
                                         start=(kk == 0), stop=(kk == M_SEM - 1))
                    hh = pff7.tile([P, 512], BF16, tag="hh", bufs=3)
                    nc.vector.tensor_tensor(hh[:], ps[:],
                                            deq1_bc[:, n * 512:(n + 1) * 512],
                                            op=ALU.mult)
                    sn = pff7.tile([P, 512], BF16, tag="snake_sin", bufs=3)
                    nc.scalar.activation(sn[:], hh[:], AF.Sin,
                                         scale=alpha_sb[:, m:m + 1])
                    sq2 = pff7.tile([P, 512], BF16, tag="snake_sq", bufs=3)
                    nc.scalar.activation(sq2[:], sn[:], AF.Square)
                    nc.vector.scalar_tensor_tensor(
                        out=h_sb[:, m, n * 512:(n + 1) * 512], in0=sq2[:],
                        scalar=rbeta_sb[:, m:m + 1], in1=hh[:],
                        op0=ALU.mult, op1=ALU.add)
        es_xq.close()   # xq/bc freed

        if debug_outs:
            nc.sync.dma_start(dbg["dbg_h"].rearrange("(m p) t -> p m t", p=P),
                              h_sb[:])

        # ================= phase 8: act_quant(h) in place =================
        with tc.tile_pool(name="pff8", bufs=1) as pff8:
            mx2 = pff8.tile([P, TOK], BF16, tag="mx2")
            mn2 = pff8.tile([P, TOK], BF16, tag="mn2")
            nc.vector.tensor_tensor(mx2[:], h_sb[:, 0, :], h_sb[:, 1, :], op=ALU.max)
            nc.vector.tensor_tensor(mn2[:], h_sb[:, 0, :], h_sb[:, 1, :], op=ALU.min)
            for m in range(2, M_FF):
                nc.vector.tensor_tensor(mx2[:], mx2[:], h_sb[:, m, :], op=ALU.max)
                nc.vector.tensor_tensor(mn2[:], mn2[:], h_sb[:, m, :], op=ALU.min)
            am2 = pff8.tile([P, TOK], BF16, tag="amax2")
            nc.vector.scalar_tensor_tensor(out=am2[:], in0=mn2[:], scalar=-1.0,
                                           in1=mx2[:], op0=ALU.mult, op1=ALU.max)
            amc2 = pff8.tile([P, TOK], F32, tag="amc2")
            nc.gpsimd.partition_all_reduce(amc2[:], am2[:], P,
                                           bass_isa.ReduceOp.absmax)
            nc.vector.tensor_scalar(amc2[:], amc2[:], 1e-5, None, ALU.max)
            sh_bc = ph.tile([P, TOK], F32)
            deq2_bc = ph.tile([P, TOK], F32)
            nc.vector.reciprocal_approx_fast(sh_bc[:], amc2[:])
            nc.vector.tensor_scalar(sh_bc[:], sh_bc[:], 127.0, None, ALU.mult)
            nc.vector.tensor_scalar(deq2_bc[:], amc2[:], mw2, 1.0 / 127.0,
                                    ALU.mult, ALU.mult)

            for m in range(M_FF):
                t = pff8.tile([P, TOK], F32, tag="hqt", bufs=2)
                nc.vector.tensor_tensor(t[:], h_sb[:, m, :], sh_bc[:], op=ALU.mult)
                nc.vector.tensor_scalar(h_sb[:, m, :], t[:], C_RND, C_RND, ALU.add,
                                        ALU.subtract)
        if debug_outs:
            nc.sync.dma_start(dbg["dbg_hq"].rearrange("(m p) t -> p m t", p=P),
                              h_sb[:])

        # ================= phase 9: FFN2 + residual -> outT =================
        with tc.tile_pool(name="pff9", bufs=1) as pff9:
            w2r = w2T.rearrange("(kt p) o -> p kt o", p=P)
            for m in range(M_SEM):
                w2q = pff9.tile([P, M_FF, P], BF16, tag="w2q", bufs=2)
                for sub in range(4):
                    wc2 = pff9.tile([P, M_SEM, P], F32, tag="w2c", bufs=2)
                    nc.sync.dma_start(
                        wc2[:], w2r[:, sub * M_SEM:(sub + 1) * M_SEM,
                                    m * P:(m + 1) * P])
                    tw2 = pff9.tile([P, M_SEM * P], F32, tag="terntmp2", bufs=2)
                    wcf2 = wc2[:].rearrange("p a b -> p (a b)")
                    nc.gpsimd.tensor_scalar(tw2[:], wcf2, sw2_bc, 1.49, ALU.mult,
                                            ALU.min)
                    nc.gpsimd.tensor_scalar(tw2[:], tw2[:], -1.49, C_RND, ALU.max,
                                            ALU.add)
                    nc.gpsimd.tensor_scalar(
                        w2q[:, sub * M_SEM:(sub + 1) * M_SEM, :]
                        .rearrange("p a b -> p (a b)"),
                        tw2[:], C_RND, None, ALU.subtract)
                for n in range(NT_Q):
                    ps = ps_mm.tile([P, 512], F32, tag="mm")
                    for kk in range(M_FF):
                        nc.tensor.matmul(ps[:], w2q[:, kk, :],
                                         h_sb[:, kk, n * 512:(n + 1) * 512],
                                         start=(kk == 0), stop=(kk == M_FF - 1))
                    t = pff9.tile([P, 512], F32, tag="yev", bufs=2)
                    nc.vector.tensor_tensor(t[:], ps[:],
                                            deq2_bc[:, n * 512:(n + 1) * 512],
                                            op=ALU.mult)
                    yo = pff9.tile([P, 512], F32, tag="yout", bufs=3)
                    nc.vector.tensor_tensor(yo[:], t[:],
                                            semout_sb[:, m, n * 512:(n + 1) * 512],
                                            op=ALU.add)
                    nc.sync.dma_start(outT[m * P:(m + 1) * P, n * 512:(n + 1) * 512],
                                      yo[:])
        es_h.close()
        es_so.close()

    nc.compile()
    return nc


_NC_CACHE = {}


def _get_nc(debug_outs=False):
    key = bool(debug_outs)
    if key not in _NC_CACHE:
        _NC_CACHE[key] = build_nc(debug_outs)
    return _NC_CACHE[key]


def make_in_maps(inputs):
    """Host-side shard + layout prep. inputs: dict of full np arrays."""
    import ml_dtypes
    bf = ml_dtypes.bfloat16
    f32 = np.float32
    sem = np.asarray(inputs["sem"], f32)
    pro = np.asarray(inputs["pro"], f32)

    def cols(v, nm):
        return np.ascontiguousarray(np.asarray(v, f32).reshape(nm, P).T)

    common = {
        "gsem": cols(inputs["g_sem"], M_SEM),
        "gpro": cols(inputs["g_pro"], M_PRO),
        "gff": cols(inputs["g_ff"], M_SEM),
        "bq": cols(inputs["bq"], M_SEM),
        "bk": cols(inputs["bk"], M_SEM),
        "bv": cols(inputs["bv"], M_SEM),
        "bo": cols(inputs["bo"], M_SEM),
        "alpha": cols(inputs["alpha"], M_FF),
        "beta": cols(inputs["beta"], M_FF),
        "w1T": np.ascontiguousarray(np.asarray(inputs["W1"], f32).T),
        "w2T": np.ascontiguousarray(np.asarray(inputs["W2"], f32).T),
        "wqT": np.ascontiguousarray(np.asarray(inputs["Wq"], f32).T).astype(bf),
        "wkT": np.ascontiguousarray(np.asarray(inputs["Wk"], f32).T).astype(bf),
        "wvT": np.ascontiguousarray(np.asarray(inputs["Wv"], f32).T).astype(bf),
        "woT": np.ascontiguousarray(np.asarray(inputs["Wo"], f32).T).astype(bf),
    }

    in_maps = []
    for c in range(N_CORES):
        b, half = c // 2, c % 2
        m = dict(common)
        m["semT"] = np.ascontiguousarray(sem[b, half * TOK:(half + 1) * TOK, :].T)
        m["proT"] = np.ascontiguousarray(pro[b].T)
        m["w1s"] = np.ascontiguousarray(common["w1T"][c * P:(c + 1) * P, :])
        m["w2s"] = np.ascontiguousarray(common["w2T"][c * DP:(c + 1) * DP, :])
        in_maps.append(m)
    return in_maps


def assemble_out(results):
    out = np.empty((B, S, DS), np.float32)
    for c in range(N_CORES):
        b, half = c // 2, c % 2
        out[b, half * TOK:(half + 1) * TOK, :] = results[c]["outT"].T
    return out


def kernel(**inputs):
    nc = _get_nc()
    in_maps = make_in_maps(inputs)
    res = run_bass_kernel_spmd(nc, in_maps, core_ids=list(range(N_CORES)))
    return assemble_out(res.results)
